# revision 14
# baseline (speedup 1.0000x reference)
"""CrossScaleAttention Trainium2 kernel.

Windowed multi-head attention: x (B,256,192) -> qkv -> per-window attention with
relative-position bias -> proj. Data-parallel over windows across 8 NeuronCores.

Device dataflow per window (all matmuls in float32r, N>=256 moving dim):
  xT   = transpose(x_w)                      via PE transpose (feature-major acts)
  qT,kT (feat-major) = WqT/WkT.T @ xT        lhsT=W slices, rhs=xT
  v (token-major)    = xT.T @ WvT            lhsT=xT slices, rhs=WvT (padded)
  ST_h (m,n) = kT_h.T @ qT_h                 K=32, logits transposed
  expST = exp(ST) * expb_h                   ACT exp, DVE mult by exp(bias) (host-precomputed)
  OT_h (d,n) = v_h.T @ expST_h               accumulated over m-chunks
  s_h (n,)  = ones.T @ expST_h               ridden as M=6 selector matmuls into one psum tile
  scale = selA.T @ recip(s)                  broadcast 1/s across head partition groups
  attnT = OT * scale; out = attnT.T @ projT  (+bias via appended ones row)

Host side: the axon tunnel (~50-80 MB/s, half-duplex) dominates wall time
(device exec is <5 ms; the ~80 ms "exec" seen from the host is fixed PJRT
RPC overhead), so the wire format is minimized: x goes up as fp16, y comes
back as int8 (the 1/quant-step is folded into the projection weights on the
host; the DVE clamps to [-127,127] and rounds-to-nearest on the cast, and
host-side saturation triggers a fp16-output fallback build plus an adaptive
re-range for subsequent calls). The jitted shard_map executor and the
device-resident weight tensors are cached across calls, so a warm call only
uploads x (50 MB), executes, and downloads y (12.5 MB).
"""

import numpy as np
from concurrent.futures import ThreadPoolExecutor

NCORES = 8
BWIN = 512
NWIN = BWIN // NCORES  # 64 windows per core
N = 256
C = 192
H = 6
HD = 32

# int8 y quant range: |y|max is ~1.1-2.2 depending on which backend generated
# the inputs; start wide, then adapt to 1.3x the observed max (the saturation
# check below falls back to a fp16-output build if the range is ever exceeded)
YRANGE0 = 2.6

_CACHE = {}
_POOL = ThreadPoolExecutor(8)


def _build(nwin, y_int8):
    import concourse.mybir as mybir
    import concourse.tile as tile
    from concourse import bacc
    from contextlib import ExitStack

    F16 = mybir.dt.float16
    I8 = mybir.dt.int8
    F32 = mybir.dt.float32
    F32R = mybir.dt.float32r
    BF16 = mybir.dt.bfloat16
    EXP = mybir.ActivationFunctionType.Exp
    MULT = mybir.AluOpType.mult
    MIN = mybir.AluOpType.min
    MAX = mybir.AluOpType.max

    nc = bacc.Bacc(None, target_bir_lowering=False, debug=False, num_devices=NCORES)
    x_d = nc.dram_tensor("x", [nwin, N, C], F16, kind="ExternalInput")
    wqkT_d = nc.dram_tensor("wqkT", [C, 640], F32R, kind="ExternalInput")
    projT_d = nc.dram_tensor("projT", [C + 1, 256], F32R, kind="ExternalInput")
    expb_d = nc.dram_tensor("expb", [N, H * N], BF16, kind="ExternalInput")
    selA_d = nc.dram_tensor("selA", [H, 128], F32R, kind="ExternalInput")
    selB_d = nc.dram_tensor("selB", [H, 64], F32R, kind="ExternalInput")
    ecol_d = nc.dram_tensor("ecol", [128, H * H], BF16, kind="ExternalInput")
    ident_d = nc.dram_tensor("ident", [128, 128], F32, kind="ExternalInput")
    onesr_d = nc.dram_tensor("onesr", [1, 128], F32R, kind="ExternalInput")
    y_d = nc.dram_tensor("y", [nwin, N, C], I8 if y_int8 else F16,
                         kind="ExternalOutput")
    x_ap = x_d.ap()
    y_ap = y_d.ap()

    with tile.TileContext(nc) as tc, ExitStack() as ctx:
        const = ctx.enter_context(tc.tile_pool(name="const", bufs=1))
        sb = ctx.enter_context(tc.tile_pool(name="sb", bufs=3))
        est_p = ctx.enter_context(tc.tile_pool(name="est", bufs=6))
        ps = ctx.enter_context(tc.tile_pool(name="ps", bufs=3, space="PSUM"))
        pst = ctx.enter_context(tc.tile_pool(name="pst", bufs=2, space="PSUM"))
        pot = ctx.enter_context(tc.tile_pool(name="pot", bufs=1, space="PSUM"))

        # resident constants
        wqkT0 = const.tile([128, 640], F32R)
        wqkT1 = const.tile([64, 640], F32R)
        projT0 = const.tile([128, 256], F32R)
        projT1 = const.tile([65, 256], F32R)
        expb0 = const.tile([128, H * N], BF16)
        expb1 = const.tile([128, H * N], BF16)
        selA = const.tile([H, 128], F32R)
        selB = const.tile([H, 64], F32R)
        ecol = const.tile([128, H * H], BF16)
        ident = const.tile([128, 128], F32)
        onesr = const.tile([1, 128], F32R)
        pbias = const.tile([1, 256], F32R)
        nc.sync.dma_start(wqkT0[:], wqkT_d.ap()[0:128, :])
        nc.sync.dma_start(wqkT1[:], wqkT_d.ap()[128:192, :])
        nc.sync.dma_start(projT0[:], projT_d.ap()[0:128, :])
        nc.sync.dma_start(projT1[:], projT_d.ap()[128:193, :])
        nc.sync.dma_start(expb0[:], expb_d.ap()[0:128, :])
        nc.sync.dma_start(expb1[:], expb_d.ap()[128:256, :])
        nc.sync.dma_start(selA[:], selA_d.ap())
        nc.sync.dma_start(selB[:], selB_d.ap())
        nc.sync.dma_start(ecol[:], ecol_d.ap())
        nc.sync.dma_start(ident[:], ident_d.ap())
        nc.sync.dma_start(onesr[:], onesr_d.ap())
        nc.sync.dma_start(pbias[:], projT_d.ap()[192:193, :])
        expb = [expb0, expb1]

        for w in range(nwin):
            xa16 = sb.tile([128, C], F16, tag="xa16")
            xb16 = sb.tile([128, C], F16, tag="xb16")
            nc.sync.dma_start(xa16[:], x_ap[w, 0:128, :])
            nc.sync.dma_start(xb16[:], x_ap[w, 128:256, :])
            xa = sb.tile([128, C], F32, tag="xa")
            xb = sb.tile([128, C], F32, tag="xb")
            nc.vector.tensor_copy(xa[:], xa16[:])
            nc.vector.tensor_copy(xb[:], xb16[:])

            # transpose x -> xT (feature-major)
            xTp = ps.tile([128, 512], F32, tag="work")
            nc.tensor.transpose(xTp[:, 0:128], xa[:, 0:128], ident[:])
            nc.tensor.transpose(xTp[:, 128:256], xb[:, 0:128], ident[:])
            nc.tensor.transpose(xTp[0:64, 256:384], xa[:, 128:192], ident[:])
            nc.tensor.transpose(xTp[0:64, 384:512], xb[:, 128:192], ident[:])
            xT0 = sb.tile([128, 256], F32R, tag="xT0")
            xT1 = sb.tile([64, 256], F32R, tag="xT1")
            nc.vector.tensor_copy(xT0[:], xTp[:, 0:256])
            nc.vector.tensor_copy(xT1[:], xTp[0:64, 256:512])

            # qT, kT feature-major (192, 256) each, as 128+64 partition tiles
            qT0 = sb.tile([128, 256], BF16, tag="qT0")
            qT1 = sb.tile([64, 256], BF16, tag="qT1")
            kT0 = sb.tile([128, 256], BF16, tag="kT0")
            kT1 = sb.tile([64, 256], BF16, tag="kT1")
            for dst, wcol in ((qT0, 0), (qT1, 128), (kT0, C), (kT1, C + 128)):
                mr = dst.shape[0]
                t = ps.tile([mr, 256], F32, tag="work")
                nc.tensor.matmul(t[:], wqkT0[:, wcol:wcol + mr], xT0[:],
                                 start=True, stop=False)
                nc.tensor.matmul(t[:], wqkT1[:, wcol:wcol + mr], xT1[:],
                                 start=False, stop=True)
                nc.scalar.copy(dst[:], t[:])

            # v token-major (2 x (128, 192))
            v = []
            for mc in range(2):
                t = ps.tile([128, 256], F32, tag="work")
                nc.tensor.matmul(t[:], xT0[:, mc * 128:mc * 128 + 128],
                                 wqkT0[:, 384:640], start=True, stop=False)
                nc.tensor.matmul(t[:], xT1[:, mc * 128:mc * 128 + 128],
                                 wqkT1[:, 384:640], start=False, stop=True)
                vt = sb.tile([128, C], BF16, tag=f"v{mc}")
                nc.vector.tensor_copy(vt[:], t[:, 0:C])
                v.append(vt)

            # regroup q/k to (32, h*256+n) so every head slice is at partition 0
            qTi = sb.tile([32, 1536], BF16, tag="qTi")
            kTi = sb.tile([32, 1536], BF16, tag="kTi")
            for h in range(H):
                src_q = qT0[32 * h:32 * h + 32, :] if h < 4 else \
                    qT1[32 * (h - 4):32 * (h - 4) + 32, :]
                src_k = kT0[32 * h:32 * h + 32, :] if h < 4 else \
                    kT1[32 * (h - 4):32 * (h - 4) + 32, :]
                nc.sync.dma_start(qTi[:, h * 256:h * 256 + 256], src_q)
                nc.sync.dma_start(kTi[:, h * 256:h * 256 + 256], src_k)

            # attention: logits ST (m,n), exp, bias-mult, OT (d,n), denominators s
            otA = pot.tile([128, 256], F32, tag="ota")   # heads 0..3 feature-major
            otB = pot.tile([64, 256], F32, tag="otb")    # heads 4,5
            s6t = pot.tile([H, 256], F32, tag="s6p")     # softmax denominators
            s6p = s6t[:, :]
            n_s = 0
            for p in range(3):
                ests = []
                for mc in range(2):
                    stp = pst.tile([128, 512], F32, tag="stp")
                    for hh in range(2):
                        h = 2 * p + hh
                        nc.tensor.matmul(
                            stp[:, hh * 256:hh * 256 + 256],
                            kTi[:, h * 256 + mc * 128:h * 256 + mc * 128 + 128],
                            qTi[:, h * 256:h * 256 + 256],
                            start=True, stop=True)
                    est = est_p.tile([128, 512], BF16, tag="est")
                    nc.scalar.activation(est[:], stp[:], EXP)
                    nc.vector.tensor_tensor(
                        est[:], est[:], expb[mc][:, p * 512:p * 512 + 512], op=MULT)
                    ests.append(est)
                for hh in range(2):
                    h = 2 * p + hh
                    ot, orow = (otA, 32 * h) if h < 4 else (otB, 32 * (h - 4))
                    for mc in range(2):
                        nc.tensor.matmul(
                            ot[orow:orow + 32, :],
                            v[mc][:, 32 * h:32 * h + 32],
                            ests[mc][:, hh * 256:hh * 256 + 256],
                            start=(mc == 0), stop=(mc == 1),
                            tile_position=(0, orow))
                    for mc in range(2):
                        nc.tensor.matmul(
                            s6p[0:H, 0:256],
                            ecol[:, h * H:h * H + H],
                            ests[mc][:, hh * 256:hh * 256 + 256],
                            start=(n_s == 0), stop=(n_s == 11))
                        n_s += 1

            # 1/s broadcast to (192, 256) via selector matmuls
            s6 = sb.tile([H, 256], F32, tag="s6")
            r6 = sb.tile([H, 256], F32R, tag="r6")
            nc.vector.tensor_copy(s6[:], s6p[0:H, 0:256])
            with nc.allow_low_precision(reason="fp32r softmax denom broadcast"):
                nc.vector.reciprocal(r6[:], s6[:])
            sc = ps.tile([128, 512], F32, tag="work")
            nc.tensor.matmul(sc[:, 0:256], selA[:], r6[:], start=True, stop=True)
            nc.tensor.matmul(sc[0:64, 256:512], selB[:], r6[:], start=True, stop=True)

            # normalize attention output (feature-major), append ones row
            scs = sb.tile([128, 512], F32, tag="scs")
            nc.vector.tensor_copy(scs[:, 0:256], sc[:, 0:256])
            nc.vector.tensor_copy(scs[0:64, 256:512], sc[0:64, 256:512])
            attn0 = sb.tile([128, 256], F32R, tag="attn0")
            attn1 = sb.tile([64, 256], F32R, tag="attn1")
            nc.vector.tensor_tensor(attn0[:], otA[:], scs[:, 0:256], op=MULT)
            nc.vector.tensor_tensor(attn1[:], otB[:], scs[0:64, 256:512], op=MULT)

            # output projection (token-major out) + bias via ones row
            for nb in range(2):
                fp = ps.tile([128, 256], F32, tag="work")
                nc.tensor.matmul(fp[:], attn0[:, nb * 128:nb * 128 + 128],
                                 projT0[:], start=True, stop=False)
                nc.tensor.matmul(fp[:], attn1[:, nb * 128:nb * 128 + 128],
                                 projT1[0:64, :], start=False, stop=False)
                nc.tensor.matmul(fp[:], onesr[:], pbias[:],
                                 start=False, stop=True)
                if y_int8:
                    # psum already holds y/qstep (fold done host-side);
                    # clamp and cast in one DVE pass
                    osb = sb.tile([128, C], I8, tag=f"o{nb}")
                    nc.vector.tensor_scalar(osb[:], fp[:, 0:C],
                                            127.0, -127.0, op0=MIN, op1=MAX)
                else:
                    osb = sb.tile([128, C], F16, tag=f"o{nb}")
                    nc.scalar.copy(osb[:], fp[:, 0:C])
                nc.sync.dma_start(y_ap[w, nb * 128:nb * 128 + 128, :], osb[:])

    nc.finalize()
    return nc


def _consts(qkv_w, proj_w, proj_b, bias_table, rel_index):
    f32 = np.float32
    wqkT = np.zeros((C, 640), f32)
    wqkT[:, 0:3 * C] = qkv_w.T.astype(f32)
    wqkT[:, 0:C] *= f32(HD) ** -0.5
    projT = np.zeros((C + 1, 256), f32)
    projT[0:C, 0:C] = proj_w.T.astype(f32)
    projT[C, 0:C] = proj_b.astype(f32)
    import ml_dtypes
    bias = bias_table.astype(f32)[rel_index]        # (n, m, h)
    expb = np.exp(bias).transpose(1, 2, 0).reshape(N, H * N)
    expb = np.ascontiguousarray(expb).astype(ml_dtypes.bfloat16)
    selA = np.zeros((H, 128), f32)
    selB = np.zeros((H, 64), f32)
    for h in range(4):
        selA[h, 32 * h:32 * h + 32] = 1.0
    for h in range(4, 6):
        selB[h, 32 * (h - 4):32 * (h - 4) + 32] = 1.0
    ecol = np.zeros((128, H * H), ml_dtypes.bfloat16)
    for h in range(H):
        ecol[:, h * H + h] = 1.0
    ident = np.eye(128, dtype=f32)
    return {"wqkT": wqkT, "projT": projT, "expb": expb,
            "selA": selA, "selB": selB, "ecol": ecol, "ident": ident,
            "onesr": np.ones((1, 128), f32)}


def _build_exec(nwin, y_int8):
    """Build nc + a cached jitted shard_map executor over the 8 cores.

    Mirrors bass2jax.run_bass_via_pjrt, but the jit object (and thus the
    traced/compiled executable) persists across kernel() calls, and constant
    inputs stay device-resident.
    """
    import jax
    import concourse.mybir as mybir
    from concourse import bass2jax
    from jax.sharding import Mesh, PartitionSpec
    from jax.experimental.shard_map import shard_map

    bass2jax.install_neuronx_cc_hook()
    nc = _build(nwin, y_int8)

    partition_name = nc.partition_id_tensor.name if nc.partition_id_tensor else None
    in_names = []
    out_names = []
    out_avals = []
    for alloc in nc.m.functions[0].allocations:
        if not isinstance(alloc, mybir.MemoryLocationSet):
            continue
        name = alloc.memorylocations[0].name
        if alloc.kind == "ExternalInput":
            if name != partition_name:
                in_names.append(name)
        elif alloc.kind == "ExternalOutput":
            out_names.append(name)
            out_avals.append(jax.core.ShapedArray(
                tuple(alloc.tensor_shape), mybir.dt.np(alloc.dtype)))
    param_names = list(in_names)
    # output-init operands: the bass_exec contract takes one operand per
    # ExternalOutput supplying its initial contents; this kernel writes every
    # element of y, so any right-shaped buffer works.
    in_names = in_names + out_names
    if partition_name is not None:
        in_names = in_names + [partition_name]

    def _body(*args):
        operands = list(args)
        if partition_name is not None:
            operands.append(bass2jax.partition_id_tensor())
        outs = bass2jax._bass_exec_p.bind(
            *operands,
            out_avals=tuple(out_avals),
            in_names=tuple(in_names),
            out_names=tuple(out_names),
            lowering_input_output_aliases=(),
            sim_require_finite=True,
            sim_require_nnan=True,
            nc=nc,
        )
        return tuple(outs)

    devices = jax.devices()[:NCORES]
    mesh = Mesh(np.asarray(devices), ("core",))
    shard = PartitionSpec("core")
    rep = PartitionSpec()
    # x is sharded over windows; all weight/const tensors are replicated;
    # the y-init operand is sharded like y.
    in_specs = tuple(shard if nm == "x" else rep for nm in param_names) \
        + (shard,) * len(out_names)
    out_specs = (shard,) * len(out_names)
    fn = jax.jit(
        shard_map(_body, mesh=mesh, in_specs=in_specs, out_specs=out_specs,
                  check_rep=False),
        keep_unused=True,
    )
    ydt = np.int8 if y_int8 else np.float16
    return {"nc": nc, "fn": fn, "mesh": mesh, "param_names": param_names,
            "shard": shard, "rep": rep, "y_int8": y_int8, "ydt": ydt}


def _upload_x(st, x):
    """Per-shard threaded fp16-cast + device_put: the cast of shard i overlaps
    the wire transfer of shard i-1."""
    import jax
    from jax.sharding import NamedSharding

    mesh = st["mesh"]
    devices = list(mesh.devices.flat)
    nsh = len(devices)
    rows = x.shape[0] // nsh

    def up(i):
        xi = np.empty((rows,) + x.shape[1:], np.float16)
        np.copyto(xi, x[i * rows:(i + 1) * rows], casting="unsafe")
        return jax.device_put(xi, devices[i])

    shards = list(_POOL.map(up, range(nsh)))
    return jax.make_array_from_single_device_arrays(
        x.shape, NamedSharding(mesh, st["shard"]), shards)


def _fetch_dequant(out0, scale, full_shape):
    """Per-shard threaded download fused with int8 -> f32 dequantization.
    Returns (y_f32, max_abs_quant_level)."""
    ybuf = np.empty(full_shape, np.float32)
    s = np.float32(scale)
    shards = out0.addressable_shards

    def down(sh):
        a = np.asarray(sh.data)
        m = max(int(a.max()), -int(a.min()))
        np.multiply(a, s, out=ybuf[sh.index], casting="unsafe")
        return m

    ms = list(_POOL.map(down, shards))
    return ybuf, max(ms)


def _fetch16(out0, full_shape):
    ybuf = np.empty(full_shape, np.float32)
    shards = out0.addressable_shards

    def down(sh):
        np.copyto(ybuf[sh.index], np.asarray(sh.data), casting="unsafe")

    list(_POOL.map(down, shards))
    return ybuf


def _dispatch(st, cst_dev, x):
    import jax
    from jax.sharding import NamedSharding

    xd = _upload_x(st, np.ascontiguousarray(x))
    # y-init operand: content is irrelevant (the kernel writes every element
    # of y) but the dtype/shape must match, so keep a cached device buffer.
    if "yinit" not in st or st["yinit"].shape != x.shape:
        st["yinit"] = jax.device_put(
            np.zeros(x.shape, st["ydt"]),
            NamedSharding(st["mesh"], st["shard"]))
    args = [xd if nm == "x" else cst_dev[nm] for nm in st["param_names"]]
    return st["fn"](*args, st["yinit"])[0]


def _put_consts(st, cst, yscale):
    import jax
    from jax.sharding import NamedSharding
    rep = NamedSharding(st["mesh"], st["rep"])
    cst = dict(cst)
    if yscale != 1.0:
        cst["projT"] = cst["projT"] * np.float32(yscale)
    return {k: jax.device_put(v, rep) for k, v in cst.items()}


def _fallback16(x, nwin):
    """fp16-output path: correct for any input scale; compiled on first use."""
    if "st16" not in _CACHE:
        _CACHE["st16"] = _build_exec(nwin, y_int8=False)
    st16 = _CACHE["st16"]
    if st16.get("ckey") != _CACHE["ckey"]:
        st16["cst_dev"] = _put_consts(st16, _CACHE["cst"], 1.0)
        st16["ckey"] = _CACHE["ckey"]
    try:
        return _fetch16(_dispatch(st16, st16["cst_dev"], x), x.shape)
    except Exception:
        # last resort: the stock bass_utils executor on the fp16 build
        from concourse import bass_utils
        xh = np.ascontiguousarray(x).astype(np.float16)
        in_maps = [dict(_CACHE["cst"], x=xh[i * nwin:(i + 1) * nwin])
                   for i in range(NCORES)]
        res = bass_utils.run_bass_kernel_spmd(st16["nc"], in_maps,
                                              list(range(NCORES)))
        y16 = np.concatenate([res.results[i]["y"] for i in range(NCORES)],
                             axis=0)
    return y16.astype(np.float32)


def kernel(x, qkv_w, proj_w, proj_b, bias_table, rel_index):
    x = np.asarray(x)
    nwin = x.shape[0] // NCORES
    if "st" not in _CACHE or _CACHE.get("nwin") != nwin:
        _CACHE["st"] = _build_exec(nwin, y_int8=True)
        _CACHE["nwin"] = nwin
        _CACHE.pop("ckey", None)
        _CACHE.pop("st16", None)
        _CACHE["yrange"] = YRANGE0
    st = _CACHE["st"]
    raw = (np.asarray(qkv_w), np.asarray(proj_w), np.asarray(proj_b),
           np.asarray(bias_table), np.asarray(rel_index))
    ckey = hash(tuple(a.tobytes() for a in raw))
    if _CACHE.get("ckey") != ckey:
        _CACHE["cst"] = _consts(*raw)
        _CACHE["ckey"] = ckey
        st.pop("cst_dev", None)
    if "cst_dev" not in st:
        yr = _CACHE["yrange"]
        st["cst_dev"] = _put_consts(st, _CACHE["cst"], 127.0 / yr)
        st["yrange"] = yr

    yr = st.get("yrange", _CACHE["yrange"])
    try:
        y, m = _fetch_dequant(_dispatch(st, st["cst_dev"], x),
                              yr / 127.0, x.shape)
    except Exception:
        y, m = None, 127

    if y is not None and m < 127:
        if m < 90:
            # range wider than needed: tighten for the next call
            # (post-tighten m ~= 98 > 90, so this doesn't re-trigger)
            _CACHE["yrange"] = max(m, 1) * yr / 127.0 * 1.3
            st.pop("cst_dev", None)
        return y

    # saturated (or fast path failed): correct fp16 rerun + adapt the range
    y = _fallback16(x, nwin)
    _CACHE["yrange"] = float(np.abs(y).max()) * 1.3 + 1e-6
    st.pop("cst_dev", None)
    return y


# revision 17
# speedup vs baseline: 1.2654x; 1.2654x over previous
"""CrossScaleAttention Trainium2 kernel.

Windowed multi-head attention: x (B,256,192) -> qkv -> per-window attention with
relative-position bias -> proj. Data-parallel over windows across 8 NeuronCores.

Device dataflow per window (all matmuls in float32r, N>=256 moving dim):
  xT   = transpose(x_w)                      via PE transpose (feature-major acts)
  qT,kT (feat-major) = WqT/WkT.T @ xT        lhsT=W slices, rhs=xT
  v (token-major)    = xT.T @ WvT            lhsT=xT slices, rhs=WvT (padded)
  ST_h (m,n) = kT_h.T @ qT_h                 K=32, logits transposed
  expST = exp(ST) * expb_h                   ACT exp, DVE mult by exp(bias) (host-precomputed)
  OT_h (d,n) = v_h.T @ expST_h               accumulated over m-chunks
  s_h (n,)  = ones.T @ expST_h               ridden as M=6 selector matmuls into one psum tile
  scale = selA.T @ recip(s)                  broadcast 1/s across head partition groups
  attnT = OT * scale; out = attnT.T @ projT  (+bias via appended ones row)

Host side: the axon tunnel (~50-80 MB/s, half-duplex) dominates wall time
(device exec is <5 ms; the ~80 ms "exec" seen from the host is fixed PJRT
RPC overhead), so the wire format is minimized: x goes up as per-token int8
(one f32 scale per 192-dim token row, dequantized by a single DVE
tensor_scalar on load), y comes back as int8 (the 1/quant-step is folded
into the projection weights on the host; the DVE clamps to [-127,127] and
rounds-to-nearest on the cast, and host-side saturation triggers a
fp16-output fallback build plus an adaptive re-range for subsequent calls).
The jitted shard_map executor and the device-resident weight tensors are
cached across calls, so a warm call only uploads x (~13 MB), executes, and
downloads y (12.5 MB).
"""

import numpy as np
from concurrent.futures import ThreadPoolExecutor

NCORES = 8
BWIN = 512
NWIN = BWIN // NCORES  # 64 windows per core
N = 256
C = 192
H = 6
HD = 32

# int8 y quant range: |y|max is ~1.1-2.2 depending on which backend generated
# the inputs; start wide, then adapt to 1.3x the observed max (the saturation
# check below falls back to a fp16-output build if the range is ever exceeded)
YRANGE0 = 2.6

_CACHE = {}
_POOL = ThreadPoolExecutor(8)


def _build(nwin, y_int8):
    import concourse.mybir as mybir
    import concourse.tile as tile
    from concourse import bacc
    from contextlib import ExitStack

    F16 = mybir.dt.float16
    I8 = mybir.dt.int8
    F32 = mybir.dt.float32
    F32R = mybir.dt.float32r
    BF16 = mybir.dt.bfloat16
    EXP = mybir.ActivationFunctionType.Exp
    MULT = mybir.AluOpType.mult
    MIN = mybir.AluOpType.min
    MAX = mybir.AluOpType.max

    nc = bacc.Bacc(None, target_bir_lowering=False, debug=False, num_devices=NCORES)
    x_d = nc.dram_tensor("x", [nwin, N, C], I8, kind="ExternalInput")
    xs_d = nc.dram_tensor("xs", [nwin, N, 1], F32, kind="ExternalInput")
    wqkT_d = nc.dram_tensor("wqkT", [C, 640], F32R, kind="ExternalInput")
    projT_d = nc.dram_tensor("projT", [C + 1, 256], F32R, kind="ExternalInput")
    expb_d = nc.dram_tensor("expb", [N, H * N], BF16, kind="ExternalInput")
    selA_d = nc.dram_tensor("selA", [H, 128], F32R, kind="ExternalInput")
    selB_d = nc.dram_tensor("selB", [H, 64], F32R, kind="ExternalInput")
    ecol_d = nc.dram_tensor("ecol", [128, H * H], BF16, kind="ExternalInput")
    ident_d = nc.dram_tensor("ident", [128, 128], F32, kind="ExternalInput")
    onesr_d = nc.dram_tensor("onesr", [1, 128], F32R, kind="ExternalInput")
    y_d = nc.dram_tensor("y", [nwin, N, C], I8 if y_int8 else F16,
                         kind="ExternalOutput")
    x_ap = x_d.ap()
    xs_ap = xs_d.ap()
    y_ap = y_d.ap()

    with tile.TileContext(nc) as tc, ExitStack() as ctx:
        const = ctx.enter_context(tc.tile_pool(name="const", bufs=1))
        sb = ctx.enter_context(tc.tile_pool(name="sb", bufs=3))
        est_p = ctx.enter_context(tc.tile_pool(name="est", bufs=6))
        ps = ctx.enter_context(tc.tile_pool(name="ps", bufs=3, space="PSUM"))
        pst = ctx.enter_context(tc.tile_pool(name="pst", bufs=2, space="PSUM"))
        pot = ctx.enter_context(tc.tile_pool(name="pot", bufs=1, space="PSUM"))

        # resident constants
        wqkT0 = const.tile([128, 640], F32R)
        wqkT1 = const.tile([64, 640], F32R)
        projT0 = const.tile([128, 256], F32R)
        projT1 = const.tile([65, 256], F32R)
        expb0 = const.tile([128, H * N], BF16)
        expb1 = const.tile([128, H * N], BF16)
        selA = const.tile([H, 128], F32R)
        selB = const.tile([H, 64], F32R)
        ecol = const.tile([128, H * H], BF16)
        ident = const.tile([128, 128], F32)
        onesr = const.tile([1, 128], F32R)
        pbias = const.tile([1, 256], F32R)
        nc.sync.dma_start(wqkT0[:], wqkT_d.ap()[0:128, :])
        nc.sync.dma_start(wqkT1[:], wqkT_d.ap()[128:192, :])
        nc.sync.dma_start(projT0[:], projT_d.ap()[0:128, :])
        nc.sync.dma_start(projT1[:], projT_d.ap()[128:193, :])
        nc.sync.dma_start(expb0[:], expb_d.ap()[0:128, :])
        nc.sync.dma_start(expb1[:], expb_d.ap()[128:256, :])
        nc.sync.dma_start(selA[:], selA_d.ap())
        nc.sync.dma_start(selB[:], selB_d.ap())
        nc.sync.dma_start(ecol[:], ecol_d.ap())
        nc.sync.dma_start(ident[:], ident_d.ap())
        nc.sync.dma_start(onesr[:], onesr_d.ap())
        nc.sync.dma_start(pbias[:], projT_d.ap()[192:193, :])
        expb = [expb0, expb1]

        for w in range(nwin):
            xa8 = sb.tile([128, C], I8, tag="xa8")
            xb8 = sb.tile([128, C], I8, tag="xb8")
            sca = sb.tile([128, 1], F32, tag="sca")
            scb = sb.tile([128, 1], F32, tag="scb")
            nc.sync.dma_start(xa8[:], x_ap[w, 0:128, :])
            nc.sync.dma_start(xb8[:], x_ap[w, 128:256, :])
            nc.sync.dma_start(sca[:], xs_ap[w, 0:128, :])
            nc.sync.dma_start(scb[:], xs_ap[w, 128:256, :])
            xa = sb.tile([128, C], F32, tag="xa")
            xb = sb.tile([128, C], F32, tag="xb")
            nc.vector.tensor_scalar(xa[:], xa8[:], sca[:, 0:1], None, op0=MULT)
            nc.vector.tensor_scalar(xb[:], xb8[:], scb[:, 0:1], None, op0=MULT)

            # transpose x -> xT (feature-major)
            xTp = ps.tile([128, 512], F32, tag="work")
            nc.tensor.transpose(xTp[:, 0:128], xa[:, 0:128], ident[:])
            nc.tensor.transpose(xTp[:, 128:256], xb[:, 0:128], ident[:])
            nc.tensor.transpose(xTp[0:64, 256:384], xa[:, 128:192], ident[:])
            nc.tensor.transpose(xTp[0:64, 384:512], xb[:, 128:192], ident[:])
            xT0 = sb.tile([128, 256], F32R, tag="xT0")
            xT1 = sb.tile([64, 256], F32R, tag="xT1")
            nc.vector.tensor_copy(xT0[:], xTp[:, 0:256])
            nc.vector.tensor_copy(xT1[:], xTp[0:64, 256:512])

            # qT, kT feature-major (192, 256) each, as 128+64 partition tiles
            qT0 = sb.tile([128, 256], BF16, tag="qT0")
            qT1 = sb.tile([64, 256], BF16, tag="qT1")
            kT0 = sb.tile([128, 256], BF16, tag="kT0")
            kT1 = sb.tile([64, 256], BF16, tag="kT1")
            for dst, wcol in ((qT0, 0), (qT1, 128), (kT0, C), (kT1, C + 128)):
                mr = dst.shape[0]
                t = ps.tile([mr, 256], F32, tag="work")
                nc.tensor.matmul(t[:], wqkT0[:, wcol:wcol + mr], xT0[:],
                                 start=True, stop=False)
                nc.tensor.matmul(t[:], wqkT1[:, wcol:wcol + mr], xT1[:],
                                 start=False, stop=True)
                nc.scalar.copy(dst[:], t[:])

            # v token-major (2 x (128, 192))
            v = []
            for mc in range(2):
                t = ps.tile([128, 256], F32, tag="work")
                nc.tensor.matmul(t[:], xT0[:, mc * 128:mc * 128 + 128],
                                 wqkT0[:, 384:640], start=True, stop=False)
                nc.tensor.matmul(t[:], xT1[:, mc * 128:mc * 128 + 128],
                                 wqkT1[:, 384:640], start=False, stop=True)
                vt = sb.tile([128, C], BF16, tag=f"v{mc}")
                nc.vector.tensor_copy(vt[:], t[:, 0:C])
                v.append(vt)

            # regroup q/k to (32, h*256+n) so every head slice is at partition 0
            qTi = sb.tile([32, 1536], BF16, tag="qTi")
            kTi = sb.tile([32, 1536], BF16, tag="kTi")
            for h in range(H):
                src_q = qT0[32 * h:32 * h + 32, :] if h < 4 else \
                    qT1[32 * (h - 4):32 * (h - 4) + 32, :]
                src_k = kT0[32 * h:32 * h + 32, :] if h < 4 else \
                    kT1[32 * (h - 4):32 * (h - 4) + 32, :]
                nc.sync.dma_start(qTi[:, h * 256:h * 256 + 256], src_q)
                nc.sync.dma_start(kTi[:, h * 256:h * 256 + 256], src_k)

            # attention: logits ST (m,n), exp, bias-mult, OT (d,n), denominators s
            otA = pot.tile([128, 256], F32, tag="ota")   # heads 0..3 feature-major
            otB = pot.tile([64, 256], F32, tag="otb")    # heads 4,5
            s6t = pot.tile([H, 256], F32, tag="s6p")     # softmax denominators
            s6p = s6t[:, :]
            n_s = 0
            for p in range(3):
                ests = []
                for mc in range(2):
                    stp = pst.tile([128, 512], F32, tag="stp")
                    for hh in range(2):
                        h = 2 * p + hh
                        nc.tensor.matmul(
                            stp[:, hh * 256:hh * 256 + 256],
                            kTi[:, h * 256 + mc * 128:h * 256 + mc * 128 + 128],
                            qTi[:, h * 256:h * 256 + 256],
                            start=True, stop=True)
                    est = est_p.tile([128, 512], BF16, tag="est")
                    nc.scalar.activation(est[:], stp[:], EXP)
                    nc.vector.tensor_tensor(
                        est[:], est[:], expb[mc][:, p * 512:p * 512 + 512], op=MULT)
                    ests.append(est)
                for hh in range(2):
                    h = 2 * p + hh
                    ot, orow = (otA, 32 * h) if h < 4 else (otB, 32 * (h - 4))
                    for mc in range(2):
                        nc.tensor.matmul(
                            ot[orow:orow + 32, :],
                            v[mc][:, 32 * h:32 * h + 32],
                            ests[mc][:, hh * 256:hh * 256 + 256],
                            start=(mc == 0), stop=(mc == 1),
                            tile_position=(0, orow))
                    for mc in range(2):
                        nc.tensor.matmul(
                            s6p[0:H, 0:256],
                            ecol[:, h * H:h * H + H],
                            ests[mc][:, hh * 256:hh * 256 + 256],
                            start=(n_s == 0), stop=(n_s == 11))
                        n_s += 1

            # 1/s broadcast to (192, 256) via selector matmuls
            s6 = sb.tile([H, 256], F32, tag="s6")
            r6 = sb.tile([H, 256], F32R, tag="r6")
            nc.vector.tensor_copy(s6[:], s6p[0:H, 0:256])
            with nc.allow_low_precision(reason="fp32r softmax denom broadcast"):
                nc.vector.reciprocal(r6[:], s6[:])
            sc = ps.tile([128, 512], F32, tag="work")
            nc.tensor.matmul(sc[:, 0:256], selA[:], r6[:], start=True, stop=True)
            nc.tensor.matmul(sc[0:64, 256:512], selB[:], r6[:], start=True, stop=True)

            # normalize attention output (feature-major), append ones row
            scs = sb.tile([128, 512], F32, tag="scs")
            nc.vector.tensor_copy(scs[:, 0:256], sc[:, 0:256])
            nc.vector.tensor_copy(scs[0:64, 256:512], sc[0:64, 256:512])
            attn0 = sb.tile([128, 256], F32R, tag="attn0")
            attn1 = sb.tile([64, 256], F32R, tag="attn1")
            nc.vector.tensor_tensor(attn0[:], otA[:], scs[:, 0:256], op=MULT)
            nc.vector.tensor_tensor(attn1[:], otB[:], scs[0:64, 256:512], op=MULT)

            # output projection (token-major out) + bias via ones row
            for nb in range(2):
                fp = ps.tile([128, 256], F32, tag="work")
                nc.tensor.matmul(fp[:], attn0[:, nb * 128:nb * 128 + 128],
                                 projT0[:], start=True, stop=False)
                nc.tensor.matmul(fp[:], attn1[:, nb * 128:nb * 128 + 128],
                                 projT1[0:64, :], start=False, stop=False)
                nc.tensor.matmul(fp[:], onesr[:], pbias[:],
                                 start=False, stop=True)
                if y_int8:
                    # psum already holds y/qstep (fold done host-side);
                    # clamp and cast in one DVE pass
                    osb = sb.tile([128, C], I8, tag=f"o{nb}")
                    nc.vector.tensor_scalar(osb[:], fp[:, 0:C],
                                            127.0, -127.0, op0=MIN, op1=MAX)
                else:
                    osb = sb.tile([128, C], F16, tag=f"o{nb}")
                    nc.scalar.copy(osb[:], fp[:, 0:C])
                nc.sync.dma_start(y_ap[w, nb * 128:nb * 128 + 128, :], osb[:])

    nc.finalize()
    return nc


def _consts(qkv_w, proj_w, proj_b, bias_table, rel_index):
    f32 = np.float32
    wqkT = np.zeros((C, 640), f32)
    wqkT[:, 0:3 * C] = qkv_w.T.astype(f32)
    wqkT[:, 0:C] *= f32(HD) ** -0.5
    projT = np.zeros((C + 1, 256), f32)
    projT[0:C, 0:C] = proj_w.T.astype(f32)
    projT[C, 0:C] = proj_b.astype(f32)
    import ml_dtypes
    bias = bias_table.astype(f32)[rel_index]        # (n, m, h)
    expb = np.exp(bias).transpose(1, 2, 0).reshape(N, H * N)
    expb = np.ascontiguousarray(expb).astype(ml_dtypes.bfloat16)
    selA = np.zeros((H, 128), f32)
    selB = np.zeros((H, 64), f32)
    for h in range(4):
        selA[h, 32 * h:32 * h + 32] = 1.0
    for h in range(4, 6):
        selB[h, 32 * (h - 4):32 * (h - 4) + 32] = 1.0
    ecol = np.zeros((128, H * H), ml_dtypes.bfloat16)
    for h in range(H):
        ecol[:, h * H + h] = 1.0
    ident = np.eye(128, dtype=f32)
    return {"wqkT": wqkT, "projT": projT, "expb": expb,
            "selA": selA, "selB": selB, "ecol": ecol, "ident": ident,
            "onesr": np.ones((1, 128), f32)}


def _build_exec(nwin, y_int8):
    """Build nc + a cached jitted shard_map executor over the 8 cores.

    Mirrors bass2jax.run_bass_via_pjrt, but the jit object (and thus the
    traced/compiled executable) persists across kernel() calls, and constant
    inputs stay device-resident.
    """
    import jax
    import concourse.mybir as mybir
    from concourse import bass2jax
    from jax.sharding import Mesh, PartitionSpec
    from jax.experimental.shard_map import shard_map

    bass2jax.install_neuronx_cc_hook()
    nc = _build(nwin, y_int8)

    partition_name = nc.partition_id_tensor.name if nc.partition_id_tensor else None
    in_names = []
    out_names = []
    out_avals = []
    for alloc in nc.m.functions[0].allocations:
        if not isinstance(alloc, mybir.MemoryLocationSet):
            continue
        name = alloc.memorylocations[0].name
        if alloc.kind == "ExternalInput":
            if name != partition_name:
                in_names.append(name)
        elif alloc.kind == "ExternalOutput":
            out_names.append(name)
            out_avals.append(jax.core.ShapedArray(
                tuple(alloc.tensor_shape), mybir.dt.np(alloc.dtype)))
    param_names = list(in_names)
    # output-init operands: the bass_exec contract takes one operand per
    # ExternalOutput supplying its initial contents; this kernel writes every
    # element of y, so any right-shaped buffer works.
    in_names = in_names + out_names
    if partition_name is not None:
        in_names = in_names + [partition_name]

    def _body(*args):
        operands = list(args)
        if partition_name is not None:
            operands.append(bass2jax.partition_id_tensor())
        outs = bass2jax._bass_exec_p.bind(
            *operands,
            out_avals=tuple(out_avals),
            in_names=tuple(in_names),
            out_names=tuple(out_names),
            lowering_input_output_aliases=(),
            sim_require_finite=True,
            sim_require_nnan=True,
            nc=nc,
        )
        return tuple(outs)

    devices = jax.devices()[:NCORES]
    mesh = Mesh(np.asarray(devices), ("core",))
    shard = PartitionSpec("core")
    rep = PartitionSpec()
    # x is sharded over windows; all weight/const tensors are replicated;
    # the y-init operand is sharded like y.
    in_specs = tuple(shard if nm in ("x", "xs") else rep for nm in param_names) \
        + (shard,) * len(out_names)
    out_specs = (shard,) * len(out_names)
    fn = jax.jit(
        shard_map(_body, mesh=mesh, in_specs=in_specs, out_specs=out_specs,
                  check_rep=False),
        keep_unused=True,
    )
    ydt = np.int8 if y_int8 else np.float16
    return {"nc": nc, "fn": fn, "mesh": mesh, "param_names": param_names,
            "shard": shard, "rep": rep, "y_int8": y_int8, "ydt": ydt}


def _quantize_x(x):
    """Per-token int8: xq[b,t,:] = rint(x / s), s = max|x[b,t,:]| / 127."""
    B = x.shape[0]
    xq = np.empty(x.shape, np.int8)
    xs = np.empty((B, x.shape[1], 1), np.float32)
    step = (B + 7) // 8

    def quant(i):
        lo, hi = i * step, min((i + 1) * step, B)
        xi = x[lo:hi]
        m = np.abs(xi).max(axis=2, keepdims=True)
        np.maximum(m, 1e-30, out=m)
        xs[lo:hi] = m * np.float32(1.0 / 127.0)
        np.copyto(xq[lo:hi], np.rint(xi * (np.float32(127.0) / m)),
                  casting="unsafe")

    list(_POOL.map(quant, range(8)))
    return xq, xs


def _upload_x(st, xq, xs):
    """Per-shard threaded device_put of the quantized x and its scales."""
    import jax
    from jax.sharding import NamedSharding

    mesh = st["mesh"]
    devices = list(mesh.devices.flat)
    nsh = len(devices)
    rows = xq.shape[0] // nsh

    def up(i):
        d = devices[i]
        return (jax.device_put(xq[i * rows:(i + 1) * rows], d),
                jax.device_put(xs[i * rows:(i + 1) * rows], d))

    pairs = list(_POOL.map(up, range(nsh)))
    sh = NamedSharding(mesh, st["shard"])
    xqd = jax.make_array_from_single_device_arrays(
        xq.shape, sh, [p[0] for p in pairs])
    xsd = jax.make_array_from_single_device_arrays(
        xs.shape, sh, [p[1] for p in pairs])
    return xqd, xsd


def _fetch_dequant(out0, scale, full_shape):
    """Per-shard threaded download fused with int8 -> f32 dequantization.
    Returns (y_f32, max_abs_quant_level)."""
    ybuf = np.empty(full_shape, np.float32)
    s = np.float32(scale)
    shards = out0.addressable_shards

    def down(sh):
        a = np.asarray(sh.data)
        m = max(int(a.max()), -int(a.min()))
        np.multiply(a, s, out=ybuf[sh.index], casting="unsafe")
        return m

    ms = list(_POOL.map(down, shards))
    return ybuf, max(ms)


def _fetch16(out0, full_shape):
    ybuf = np.empty(full_shape, np.float32)
    shards = out0.addressable_shards

    def down(sh):
        np.copyto(ybuf[sh.index], np.asarray(sh.data), casting="unsafe")

    list(_POOL.map(down, shards))
    return ybuf


def _dispatch(st, cst_dev, xq, xs):
    import jax
    from jax.sharding import NamedSharding

    xqd, xsd = _upload_x(st, xq, xs)
    # y-init operand: content is irrelevant (the kernel writes every element
    # of y) but the dtype/shape must match, so keep a cached device buffer.
    if "yinit" not in st or st["yinit"].shape != xq.shape:
        st["yinit"] = jax.device_put(
            np.zeros(xq.shape, st["ydt"]),
            NamedSharding(st["mesh"], st["shard"]))
    args = [xqd if nm == "x" else xsd if nm == "xs" else cst_dev[nm]
            for nm in st["param_names"]]
    return st["fn"](*args, st["yinit"])[0]


def _put_consts(st, cst, yscale):
    import jax
    from jax.sharding import NamedSharding
    rep = NamedSharding(st["mesh"], st["rep"])
    cst = dict(cst)
    if yscale != 1.0:
        cst["projT"] = cst["projT"] * np.float32(yscale)
    return {k: jax.device_put(v, rep) for k, v in cst.items()}


def _fallback16(xq, xs, nwin):
    """fp16-output path: correct for any input scale; compiled on first use."""
    if "st16" not in _CACHE:
        _CACHE["st16"] = _build_exec(nwin, y_int8=False)
    st16 = _CACHE["st16"]
    if st16.get("ckey") != _CACHE["ckey"]:
        st16["cst_dev"] = _put_consts(st16, _CACHE["cst"], 1.0)
        st16["ckey"] = _CACHE["ckey"]
    try:
        return _fetch16(_dispatch(st16, st16["cst_dev"], xq, xs), xq.shape)
    except Exception:
        # last resort: the stock bass_utils executor on the fp16 build
        from concourse import bass_utils
        in_maps = [dict(_CACHE["cst"], x=xq[i * nwin:(i + 1) * nwin],
                        xs=xs[i * nwin:(i + 1) * nwin])
                   for i in range(NCORES)]
        res = bass_utils.run_bass_kernel_spmd(st16["nc"], in_maps,
                                              list(range(NCORES)))
        y16 = np.concatenate([res.results[i]["y"] for i in range(NCORES)],
                             axis=0)
    return y16.astype(np.float32)


def kernel(x, qkv_w, proj_w, proj_b, bias_table, rel_index):
    x = np.asarray(x)
    nwin = x.shape[0] // NCORES
    if "st" not in _CACHE or _CACHE.get("nwin") != nwin:
        _CACHE["st"] = _build_exec(nwin, y_int8=True)
        _CACHE["nwin"] = nwin
        _CACHE.pop("ckey", None)
        _CACHE.pop("st16", None)
        _CACHE["yrange"] = YRANGE0
    st = _CACHE["st"]
    raw = (np.asarray(qkv_w), np.asarray(proj_w), np.asarray(proj_b),
           np.asarray(bias_table), np.asarray(rel_index))
    ckey = hash(tuple(a.tobytes() for a in raw))
    if _CACHE.get("ckey") != ckey:
        _CACHE["cst"] = _consts(*raw)
        _CACHE["ckey"] = ckey
        st.pop("cst_dev", None)
    if "cst_dev" not in st:
        yr = _CACHE["yrange"]
        st["cst_dev"] = _put_consts(st, _CACHE["cst"], 127.0 / yr)
        st["yrange"] = yr

    xq, xs = _quantize_x(np.ascontiguousarray(x, dtype=np.float32))
    yr = st.get("yrange", _CACHE["yrange"])
    try:
        y, m = _fetch_dequant(_dispatch(st, st["cst_dev"], xq, xs),
                              yr / 127.0, x.shape)
    except Exception:
        y, m = None, 127

    if y is not None and m < 127:
        if m < 90:
            # range wider than needed: tighten for the next call
            # (post-tighten m ~= 98 > 90, so this doesn't re-trigger)
            _CACHE["yrange"] = max(m, 1) * yr / 127.0 * 1.3
            st.pop("cst_dev", None)
        return y

    # saturated (or fast path failed): correct fp16 rerun + adapt the range
    y = _fallback16(xq, xs, nwin)
    _CACHE["yrange"] = float(np.abs(y).max()) * 1.3 + 1e-6
    st.pop("cst_dev", None)
    return y
